# revision 11
# baseline (speedup 1.0000x reference)
"""Causal self-attention (B=2, T=2048, C=1024, 16 heads) on 8 trn2 NeuronCores.

Sharding: core = (batch b, head-group hg); b = core//4, hg = core%4.
Each core computes 4 heads' attention for one batch plus its partial output
projection (contracting only its 256 head-dims); the host sums the 4 partial
projections per batch and adds b_proj.

Per-core device program (all matmuls in float32r: full fp32 storage,
~13-bit-mantissa multiplies, 4x the fp32 matmul rate at N>=256):

  per t-slice of 512 tokens:
    phase 1: x^T tiles via PE transpose; Q^T,K^T = (W_q|k/8)^T x^T in
             [wcol, T] layout; V in natural [T, vcol] layout packed as
             V_aug[t, 65h+j] with a ones column (j=64) per head.
    phase 2 (per head-pair, q-tile = this t-slice): row-tiled concurrent
             K=64 matmuls put s^T[k,q] for both heads in one 2-bank PSUM
             tile; one exp on ACT (no max subtraction: scores ~ N(0,1));
             causal mask via one broadcast multiply with a slice of a wide
             triangular mask; y~^T[65,512] += V_aug(kt).T @ exp(s^T) with
             row 64 = softmax denominator; y^T = y~^T[0:64] * bcast(1/den).
    phase 3: partial out[t, c] = sum_m Y^T[m, t-tile].T @ W_p[m, c].
"""

import os
import sys
import types

sys.path.insert(0, "/opt/trn_rl_repo")

import numpy as np

import concourse.bass as bass  # noqa: F401
import concourse.mybir as mybir
import concourse.tile as tile
from concourse import bacc
from concourse.bass_utils import run_bass_kernel_spmd

B, T, C = 2, 2048, 1024
H, D = 16, 64
HPG = 4  # heads per core
GD = HPG * D  # 256 head-dims per core
NCORES = 8

NT128 = T // 128  # 16
NT512 = T // 512  # 4
NC128 = C // 128  # 8

F32 = mybir.dt.float32
F32R = mybir.dt.float32r
AF = mybir.ActivationFunctionType

_CACHE = {}


def _patch_act_tables():
    """Make natural_log_exp_and_others the only eligible ACT table set.

    The stock chooser greedily picks the first act_info set containing each
    activation function, so a kernel using both Exp and Ln thrashes between
    exp_and_others and natural_log (~1.3us ACT stall per switch, 17 loads).
    Emptying every other set (names/positions preserved, so the emitted
    act_func_set_id still matches walrus's act_info.json order) forces the
    combined set: one table load for the whole kernel.
    """
    import concourse.hw_specs as hw_specs
    import concourse.bacc as bacc_mod

    if getattr(hw_specs.get_activation_tables, "_patched", False):
        return
    orig = hw_specs.get_activation_tables

    def patched(arch):
        tables = orig(arch)
        return {
            name: (funcs if name == "natural_log_exp_and_others" else set())
            for name, funcs in tables.items()
        }

    patched._patched = True
    hw_specs.get_activation_tables = patched
    bacc_mod.get_activation_tables = patched


def _install_ntff_hook():
    """Register the axon NTFF profiling hook (the agent image lacks
    antenv.axon_hooks; synthesize it so trace=True works)."""
    if "antenv.axon_hooks" in sys.modules:
        return
    mod = types.ModuleType("antenv.axon_hooks")
    holder = [None]
    mod.set_axon_ntff_profile_hook = lambda h: holder.__setitem__(0, h)
    mod.get_axon_ntff_profile_hook = lambda: holder[0]
    sys.modules["antenv.axon_hooks"] = mod
    try:
        import antenv

        antenv.axon_hooks = mod
        from trn_agent_boot.trn_boot import _ntff_profile_via_ctypes

        hook = _ntff_profile_via_ctypes("/opt/axon/libaxon_pjrt.so")
        mod.set_axon_ntff_profile_hook(hook)
    except Exception:
        pass


def _build():
    _patch_act_tables()
    nc = bacc.Bacc("TRN2", target_bir_lowering=False)

    x = nc.dram_tensor("x", [T, C], F32R, kind="ExternalInput")
    wq = nc.dram_tensor("wq", [C, GD], F32R, kind="ExternalInput")
    wk = nc.dram_tensor("wk", [C, GD], F32R, kind="ExternalInput")
    wv = nc.dram_tensor("wv", [C, GD], F32R, kind="ExternalInput")
    wp = nc.dram_tensor("wp", [GD, C], F32R, kind="ExternalInput")
    bq = nc.dram_tensor("bq", [GD], F32, kind="ExternalInput")
    bk = nc.dram_tensor("bk", [GD], F32, kind="ExternalInput")
    bv = nc.dram_tensor("bv", [1, GD], F32R, kind="ExternalInput")
    ident_d = nc.dram_tensor("ident", [128, 128], F32R, kind="ExternalInput")
    maskw = nc.dram_tensor("maskw", [128, 896], F32R, kind="ExternalInput")
    ones4 = nc.dram_tensor("ones4", [128, HPG], F32R, kind="ExternalInput")
    out = nc.dram_tensor("out", [T, C], F32, kind="ExternalOutput")

    with tile.TileContext(nc) as tc:
        with (
            tc.tile_pool(name="cst", bufs=1) as cst,
            tc.tile_pool(name="big", bufs=1) as bigp,
            tc.tile_pool(name="psum", bufs=2, space="PSUM") as psum,
            tc.tile_pool(name="psacc", bufs=1, space="PSUM") as psacc,
            tc.tile_pool(name="wqkv", bufs=1) as wpool,
            tc.tile_pool(name="xn", bufs=3) as xnp,
            tc.tile_pool(name="xt", bufs=2) as xtp,
            tc.tile_pool(name="expp", bufs=3) as expp,
            tc.tile_pool(name="misc", bufs=2) as miscp,
            tc.tile_pool(name="outp", bufs=3) as outp,
        ):
            # ---- constant / weight loads (scalar-engine HWDGE queue, so x
            # loads on the sync queue aren't stuck behind them) ----
            ident = cst.tile([128, 128], F32R, tag="ident")
            nc.scalar.dma_start(ident[:], ident_d[:])
            maskw_sb = cst.tile([128, 896], F32R, tag="maskw")
            nc.scalar.dma_start(maskw_sb[:], maskw[:])
            ones_sb = cst.tile([128, HPG], F32R, tag="ones")
            nc.scalar.dma_start(ones_sb[:], ones4[:])
            wp_sb = cst.tile([128, GD // 128, C], F32R, tag="wp")
            nc.scalar.dma_start(wp_sb[:], wp[:].rearrange("(o p) c -> p o c", p=128))
            bq_sb = cst.tile([128, GD // 128], F32, tag="bq")
            nc.scalar.dma_start(bq_sb[:], bq[:].rearrange("(m p) -> p m", p=128))
            bk_sb = cst.tile([128, GD // 128], F32, tag="bk")
            nc.scalar.dma_start(bk_sb[:], bk[:].rearrange("(m p) -> p m", p=128))
            bv_row = cst.tile([1, GD], F32R, tag="bvr")
            nc.scalar.dma_start(bv_row[:], bv[:])
            bv_bc = cst.tile([128, GD], F32R, tag="bvb")
            nc.gpsimd.partition_broadcast(bv_bc[:], bv_row[:])

            wq_sb = wpool.tile([128, NC128, GD], F32R, tag="wq")
            nc.scalar.dma_start(wq_sb[:], wq[:].rearrange("(o p) c -> p o c", p=128))
            wk_sb = wpool.tile([128, NC128, GD], F32R, tag="wk")
            nc.scalar.dma_start(wk_sb[:], wk[:].rearrange("(o p) c -> p o c", p=128))
            wv_sb = wpool.tile([128, NC128, GD], F32R, tag="wv")
            nc.scalar.dma_start(wv_sb[:], wv[:].rearrange("(o p) c -> p o c", p=128))

            # persistent activations
            qt_sb = bigp.tile([128, 2, T], F32R, tag="qt")  # Q^T
            kt_sb = bigp.tile([128, 2, T], F32R, tag="kt")  # K^T
            va_sb = bigp.tile([128, NT128, 65 * HPG], F32R, tag="va")  # V_aug
            yt_sb = bigp.tile([128, 2, T], F32R, tag="yt")  # normalized Y^T

            def issue_xn(ts):
                t0 = 512 * ts
                xh = []
                for half in range(2):
                    h0 = t0 + 256 * half
                    xnh = xnp.tile([128, 2, C], F32R, tag="xn")
                    nc.sync.dma_start(
                        xnh[:],
                        x[h0 : h0 + 256, :].rearrange("(a p) c -> p a c", p=128),
                    )
                    xh.append(xnh)
                return xh

            def phase1(ts, xh):
                t0 = 512 * ts
                xt = xtp.tile([128, NC128, 512], F32R, tag="xt")
                for ci in range(NC128):
                    pt = psum.tile([128, 512], F32R, tag="work")
                    for tt in range(4):
                        nc.tensor.transpose(
                            pt[:, 128 * tt : 128 * tt + 128],
                            xh[tt // 2][:, tt % 2, 128 * ci : 128 * ci + 128],
                            ident[:],
                        )
                    nc.vector.tensor_copy(xt[:, ci, :], pt[:])

                # Q^T / K^T for this t-slice
                for m in range(4):
                    w_sb = wq_sb if m < 2 else wk_sb
                    b_sb = bq_sb if m < 2 else bk_sb
                    mm = m % 2
                    pqk = psum.tile([128, 512], F32, tag="work")
                    for ci in range(NC128):
                        nc.tensor.matmul(
                            pqk[:],
                            w_sb[:, ci, 128 * mm : 128 * mm + 128],
                            xt[:, ci, :],
                            start=(ci == 0),
                            stop=(ci == NC128 - 1),
                        )
                    dst = (qt_sb if m < 2 else kt_sb)[:, mm, t0 : t0 + 512]
                    nc.vector.tensor_scalar_add(dst, pqk[:], b_sb[:, mm : mm + 1])

                # V (natural layout) for this t-slice
                for tt in range(4):
                    kt_idx = 4 * ts + tt
                    pv = psum.tile([128, 512], F32, tag="work")
                    for ci in range(NC128):
                        nc.tensor.matmul(
                            pv[:, 0:GD],
                            xt[:, ci, 128 * tt : 128 * tt + 128],
                            wv_sb[:, ci, :],
                            start=(ci == 0),
                            stop=(ci == NC128 - 1),
                        )
                    va_t = va_sb[:, kt_idx].rearrange("p (h j) -> p h j", j=65)
                    nc.vector.tensor_tensor(
                        va_t[:, :, 0:64],
                        pv[:, 0:GD].rearrange("p (h j) -> p h j", j=64),
                        bv_bc[:].rearrange("p (h j) -> p h j", j=64),
                        mybir.AluOpType.add,
                    )
                    nc.vector.tensor_copy(va_t[:, :, 64], ones_sb[:])

            def phase2(m, qi):
                """Attention for head pair (2m, 2m+1) on q-tile qi.

                Scores for both heads are computed by two concurrent
                row-tiled K=64 matmuls (array rows 0-63 / 64-127) into the
                two banks of one [128, 1024] PSUM tile. For diagonal k-tiles
                (i = kt - 4*qi >= 1) the first 128*i q-columns are fully
                masked, so scores/exp/mask/accumulate all skip them.
                """
                q0 = 512 * qi
                nk = 4 * qi + 4
                pyA = psacc.tile([65, 512], F32, tag="pyA")
                pyB = psacc.tile([65, 512], F32, tag="pyB")

                def qlo(kt):
                    i = kt - 4 * qi
                    return 128 * i if i > 0 else 0

                pend = {}

                def emit_s(kt):
                    lo = qlo(kt)
                    ps = psum.tile([128, 1024], F32, tag="big")
                    for half in range(2):
                        po = 64 * half
                        nc.tensor.matmul(
                            ps[:, 512 * half + lo : 512 * half + 512],
                            kt_sb[po : po + 64, m, 128 * kt : 128 * kt + 128],
                            qt_sb[po : po + 64, m, q0 + lo : q0 + 512],
                            start=True,
                            stop=True,
                        )
                    pend[kt] = ps

                emit_s(0)
                if nk > 1:
                    emit_s(1)
                for kt in range(nk):
                    ps = pend.pop(kt)
                    lo = qlo(kt)
                    et = expp.tile([128, 1024], F32R, tag="exp")
                    if lo == 0:
                        nc.scalar.activation(et[:], ps[:], AF.Exp)
                    else:
                        for half in range(2):
                            sl = slice(512 * half + lo, 512 * half + 512)
                            nc.scalar.activation(et[:, sl], ps[:, sl], AF.Exp)
                    i = kt - 4 * qi
                    if i >= 0:  # diagonal tile: apply causal mask to both heads
                        o = 384 - 128 * i
                        nc.vector.tensor_tensor(
                            et[:].rearrange("p (h q) -> p h q", h=2)[:, :, lo:512],
                            et[:].rearrange("p (h q) -> p h q", h=2)[:, :, lo:512],
                            maskw_sb[:, None, o + lo : o + 512].to_broadcast(
                                [128, 2, 512 - lo]
                            ),
                            mybir.AluOpType.mult,
                        )
                    for half, py in ((0, pyA), (1, pyB)):
                        h = 2 * m + half
                        nc.tensor.matmul(
                            py[:, lo:512],
                            va_sb[:, kt, 65 * h : 65 * h + 65],
                            et[:, 512 * half + lo : 512 * half + 512],
                            start=(kt == 0),
                            stop=(kt == nk - 1),
                        )
                    if kt + 2 < nk:
                        emit_s(kt + 2)

                for half, py in ((0, pyA), (1, pyB)):
                    # 1/den = exp(-ln(den)) on ACT (DVE reciprocal is 8 cyc/elem)
                    rec = miscp.tile([1, 512], F32, tag="rec")
                    nc.scalar.activation(rec[:], py[64:65, :], AF.Ln)
                    nc.scalar.activation(rec[:], rec[:], AF.Exp, scale=-1.0)
                    bc = miscp.tile([64, 512], F32, tag="bc")
                    nc.gpsimd.partition_broadcast(bc[:], rec[:])
                    nc.vector.tensor_mul(
                        yt_sb[64 * half : 64 * half + 64, m, q0 : q0 + 512],
                        py[0:64, :],
                        bc[:],
                    )

            def phase3(tt):
                for nn in range(2):
                    po = psum.tile([128, 512], F32, tag="work")
                    for mm in range(2):
                        nc.tensor.matmul(
                            po[:],
                            yt_sb[:, mm, 128 * tt : 128 * tt + 128],
                            wp_sb[:, mm, 512 * nn : 512 * nn + 512],
                            start=(mm == 0),
                            stop=(mm == 1),
                        )
                    ot = outp.tile([128, 512], F32, tag="ot")
                    if nn == 0:
                        nc.vector.tensor_copy(ot[:], po[:])
                    else:
                        nc.scalar.copy(ot[:], po[:])
                    nc.sync.dma_start(
                        out[128 * tt : 128 * tt + 128, 512 * nn : 512 * nn + 512],
                        ot[:],
                    )

            # interleave: q-tile qi's attention only needs t-slices <= qi,
            # and the projection of t-slice ts only needs attention <= ts.
            xh_next = issue_xn(0)
            for ts in range(NT512):
                phase1(ts, xh_next)
                if ts + 1 < NT512:
                    xh_next = issue_xn(ts + 1)
                for m in range(2):
                    phase2(m, ts)
                for tt in range(4 * ts, 4 * ts + 4):
                    phase3(tt)

    nc.compile()
    return nc


def _make_maskw():
    p = np.arange(128)[:, None]
    jj = np.arange(896)[None, :]
    return (jj >= p + 384).astype(np.float32)


def kernel(x, w_qkv, b_qkv, w_proj, b_proj, _trace=False):
    x = np.asarray(x, dtype=np.float32)
    w_qkv = np.asarray(w_qkv, dtype=np.float32)
    b_qkv = np.asarray(b_qkv, dtype=np.float32)
    w_proj = np.asarray(w_proj, dtype=np.float32)
    b_proj = np.asarray(b_proj, dtype=np.float32)

    if "nc" not in _CACHE:
        _CACHE["nc"] = _build()
    nc = _CACHE["nc"]

    maskw = _make_maskw()
    ones = np.ones((128, HPG), np.float32)
    ident = np.eye(128, dtype=np.float32)
    scale = 1.0 / np.sqrt(D)

    in_maps = []
    for core in range(NCORES):
        b, hg = core // (NCORES // B), core % (NCORES // B)
        cs = slice(GD * hg, GD * hg + GD)  # this core's head columns / dims
        in_maps.append(
            {
                "x": np.ascontiguousarray(x[b]),
                "wq": np.ascontiguousarray(w_qkv[:, 0:C][:, cs]) * scale,
                "wk": np.ascontiguousarray(w_qkv[:, C : 2 * C][:, cs]),
                "wv": np.ascontiguousarray(w_qkv[:, 2 * C : 3 * C][:, cs]),
                "wp": np.ascontiguousarray(w_proj[cs, :]),
                "bq": np.ascontiguousarray(b_qkv[0:C][cs]) * scale,
                "bk": np.ascontiguousarray(b_qkv[C : 2 * C][cs]),
                "bv": np.ascontiguousarray(b_qkv[2 * C : 3 * C][cs])[None, :],
                "ident": ident,
                "maskw": maskw,
                "ones4": ones,
            }
        )

    if _trace:
        _install_ntff_hook()
    res = run_bass_kernel_spmd(
        nc, in_maps, core_ids=list(range(NCORES)), trace=bool(_trace)
    )
    _CACHE["last_result"] = res

    out = np.zeros((B, T, C), np.float32)
    for b in range(B):
        acc = res.results[4 * b + 0]["out"].astype(np.float64)
        for i in range(1, NCORES // B):
            acc += res.results[4 * b + i]["out"]
        out[b] = (acc + b_proj).astype(np.float32)
    return out


# revision 13
# speedup vs baseline: 1.4867x; 1.4867x over previous
"""Causal self-attention (B=2, T=2048, C=1024, 16 heads) on 8 trn2 NeuronCores.

Sharding: core = (batch b, head-group hg); b = core//4, hg = core%4.
Each core computes 4 heads' attention for one batch plus its partial output
projection (contracting only its 256 head-dims); the host sums the 4 partial
projections per batch and adds b_proj.

Per-core device program (all matmuls in float32r: full fp32 storage,
~13-bit-mantissa multiplies, 4x the fp32 matmul rate at N>=256):

  per t-slice of 512 tokens:
    phase 1: x^T tiles via PE transpose; Q^T,K^T = (W_q|k/8)^T x^T in
             [wcol, T] layout; V in natural [T, vcol] layout packed as
             V_aug[t, 65h+j] with a ones column (j=64) per head.
    phase 2 (per head-pair, q-tile = this t-slice): row-tiled concurrent
             K=64 matmuls put s^T[k,q] for both heads in one 2-bank PSUM
             tile; one exp on ACT (no max subtraction: scores ~ N(0,1));
             causal mask via one broadcast multiply with a slice of a wide
             triangular mask; y~^T[65,512] += V_aug(kt).T @ exp(s^T) with
             row 64 = softmax denominator; y^T = y~^T[0:64] * bcast(1/den).
    phase 3: partial out[t, c] = sum_m Y^T[m, t-tile].T @ W_p[m, c].
"""

import os
import sys
import types

sys.path.insert(0, "/opt/trn_rl_repo")

import numpy as np

import concourse.bass as bass  # noqa: F401
import concourse.mybir as mybir
import concourse.tile as tile
from concourse import bacc
from concourse.bass_utils import run_bass_kernel_spmd

B, T, C = 2, 2048, 1024
H, D = 16, 64
HPG = 4  # heads per core
GD = HPG * D  # 256 head-dims per core
NCORES = 8

NT128 = T // 128  # 16
NT512 = T // 512  # 4
NC128 = C // 128  # 8

F32 = mybir.dt.float32
F32R = mybir.dt.float32r
AF = mybir.ActivationFunctionType

_CACHE = {}


def _patch_act_tables():
    """Make natural_log_exp_and_others the only eligible ACT table set.

    The stock chooser greedily picks the first act_info set containing each
    activation function, so a kernel using both Exp and Ln thrashes between
    exp_and_others and natural_log (~1.3us ACT stall per switch, 17 loads).
    Emptying every other set (names/positions preserved, so the emitted
    act_func_set_id still matches walrus's act_info.json order) forces the
    combined set: one table load for the whole kernel.
    """
    import concourse.hw_specs as hw_specs
    import concourse.bacc as bacc_mod

    if getattr(hw_specs.get_activation_tables, "_patched", False):
        return
    orig = hw_specs.get_activation_tables

    def patched(arch):
        tables = orig(arch)
        return {
            name: (funcs if name == "natural_log_exp_and_others" else set())
            for name, funcs in tables.items()
        }

    patched._patched = True
    hw_specs.get_activation_tables = patched
    bacc_mod.get_activation_tables = patched


def _install_ntff_hook():
    """Register the axon NTFF profiling hook (the agent image lacks
    antenv.axon_hooks; synthesize it so trace=True works)."""
    if "antenv.axon_hooks" in sys.modules:
        return
    mod = types.ModuleType("antenv.axon_hooks")
    holder = [None]
    mod.set_axon_ntff_profile_hook = lambda h: holder.__setitem__(0, h)
    mod.get_axon_ntff_profile_hook = lambda: holder[0]
    sys.modules["antenv.axon_hooks"] = mod
    try:
        import antenv

        antenv.axon_hooks = mod
        from trn_agent_boot.trn_boot import _ntff_profile_via_ctypes

        hook = _ntff_profile_via_ctypes("/opt/axon/libaxon_pjrt.so")
        mod.set_axon_ntff_profile_hook(hook)
    except Exception:
        pass


def _build():
    _patch_act_tables()
    nc = bacc.Bacc("TRN2", target_bir_lowering=False)

    x = nc.dram_tensor("x", [T, C], F32R, kind="ExternalInput")
    wq = nc.dram_tensor("wq", [C, GD], F32R, kind="ExternalInput")
    wk = nc.dram_tensor("wk", [C, GD], F32R, kind="ExternalInput")
    wv = nc.dram_tensor("wv", [C, GD], F32R, kind="ExternalInput")
    wp = nc.dram_tensor("wp", [GD, C], F32R, kind="ExternalInput")
    bq = nc.dram_tensor("bq", [GD], F32, kind="ExternalInput")
    bk = nc.dram_tensor("bk", [GD], F32, kind="ExternalInput")
    bv = nc.dram_tensor("bv", [1, GD], F32R, kind="ExternalInput")
    ident_d = nc.dram_tensor("ident", [128, 128], F32R, kind="ExternalInput")
    maskw = nc.dram_tensor("maskw", [128, 896], F32R, kind="ExternalInput")
    ones4 = nc.dram_tensor("ones4", [128, HPG], F32R, kind="ExternalInput")
    out = nc.dram_tensor("out", [T, C], F32, kind="ExternalOutput")

    with tile.TileContext(nc) as tc:
        with (
            tc.tile_pool(name="cst", bufs=1) as cst,
            tc.tile_pool(name="big", bufs=1) as bigp,
            tc.tile_pool(name="psum", bufs=2, space="PSUM") as psum,
            tc.tile_pool(name="psacc", bufs=1, space="PSUM") as psacc,
            tc.tile_pool(name="wqkv", bufs=1) as wpool,
            tc.tile_pool(name="xn", bufs=3) as xnp,
            tc.tile_pool(name="xt", bufs=2) as xtp,
            tc.tile_pool(name="expp", bufs=3) as expp,
            tc.tile_pool(name="misc", bufs=2) as miscp,
            tc.tile_pool(name="outp", bufs=3) as outp,
        ):
            # ---- constant / weight loads (scalar-engine HWDGE queue, so x
            # loads on the sync queue aren't stuck behind them) ----
            ident = cst.tile([128, 128], F32R, tag="ident")
            nc.scalar.dma_start(ident[:], ident_d[:])
            maskw_sb = cst.tile([128, 896], F32R, tag="maskw")
            nc.scalar.dma_start(maskw_sb[:], maskw[:])
            ones_sb = cst.tile([128, HPG], F32R, tag="ones")
            nc.scalar.dma_start(ones_sb[:], ones4[:])
            wp_sb = cst.tile([128, GD // 128, C], F32R, tag="wp")
            nc.scalar.dma_start(wp_sb[:], wp[:].rearrange("(o p) c -> p o c", p=128))
            bq_sb = cst.tile([128, GD // 128], F32, tag="bq")
            nc.scalar.dma_start(bq_sb[:], bq[:].rearrange("(m p) -> p m", p=128))
            bk_sb = cst.tile([128, GD // 128], F32, tag="bk")
            nc.scalar.dma_start(bk_sb[:], bk[:].rearrange("(m p) -> p m", p=128))
            bv_row = cst.tile([1, GD], F32R, tag="bvr")
            nc.scalar.dma_start(bv_row[:], bv[:])
            bv_bc = cst.tile([128, GD], F32R, tag="bvb")
            nc.gpsimd.partition_broadcast(bv_bc[:], bv_row[:])

            wq_sb = wpool.tile([128, NC128, GD], F32R, tag="wq")
            nc.scalar.dma_start(wq_sb[:], wq[:].rearrange("(o p) c -> p o c", p=128))
            wk_sb = wpool.tile([128, NC128, GD], F32R, tag="wk")
            nc.scalar.dma_start(wk_sb[:], wk[:].rearrange("(o p) c -> p o c", p=128))
            wv_sb = wpool.tile([128, NC128, GD], F32R, tag="wv")
            nc.scalar.dma_start(wv_sb[:], wv[:].rearrange("(o p) c -> p o c", p=128))

            # persistent activations
            qt_sb = bigp.tile([128, 2, T], F32R, tag="qt")  # Q^T
            kt_sb = bigp.tile([128, 2, T], F32R, tag="kt")  # K^T
            va_sb = bigp.tile([128, NT128, 65 * HPG], F32R, tag="va")  # V_aug
            yt_sb = bigp.tile([128, 2, T], F32R, tag="yt")  # normalized Y^T

            def issue_xn(ts):
                t0 = 512 * ts
                xh = []
                for half in range(2):
                    h0 = t0 + 256 * half
                    xnh = xnp.tile([128, 2, C], F32R, tag="xn")
                    nc.sync.dma_start(
                        xnh[:],
                        x[h0 : h0 + 256, :].rearrange("(a p) c -> p a c", p=128),
                    )
                    xh.append(xnh)
                return xh

            def make_ph1_units(ts, xh):
                """Phase 1 for t-slice ts as a list of small PE work units
                (~2 matmuls each) that can be pumped into phase-2 PE gaps."""
                t0 = 512 * ts
                xt = xtp.tile([128, NC128, 512], F32R, tag="xt")
                units = []

                def t_unit(ci):
                    pt = psum.tile([128, 512], F32R, tag="work")
                    for tt in range(4):
                        nc.tensor.transpose(
                            pt[:, 128 * tt : 128 * tt + 128],
                            xh[tt // 2][:, tt % 2, 128 * ci : 128 * ci + 128],
                            ident[:],
                        )
                    nc.vector.tensor_copy(xt[:, ci, :], pt[:])

                for ci in range(NC128):
                    units.append(lambda ci=ci: t_unit(ci))

                def qk_sub(m, cc, box):
                    if cc == 0:
                        box["ps"] = psum.tile(
                            [128, 512], F32, tag="work", name="pqk"
                        )
                    w_sb = wq_sb if m < 2 else wk_sb
                    mm = m % 2
                    for ci in (2 * cc, 2 * cc + 1):
                        nc.tensor.matmul(
                            box["ps"][:],
                            w_sb[:, ci, 128 * mm : 128 * mm + 128],
                            xt[:, ci, :],
                            start=(ci == 0),
                            stop=(ci == NC128 - 1),
                        )
                    if cc == 3:
                        b_sb = bq_sb if m < 2 else bk_sb
                        dst = (qt_sb if m < 2 else kt_sb)[:, mm, t0 : t0 + 512]
                        nc.vector.tensor_scalar_add(
                            dst, box["ps"][:], b_sb[:, mm : mm + 1]
                        )

                for m in range(4):
                    box = {}
                    for cc in range(4):
                        units.append(lambda m=m, cc=cc, box=box: qk_sub(m, cc, box))

                def v_sub(tt, cc, box):
                    if cc == 0:
                        box["ps"] = psum.tile(
                            [128, 512], F32, tag="work", name="pv"
                        )
                    for ci in (2 * cc, 2 * cc + 1):
                        nc.tensor.matmul(
                            box["ps"][:, 0:GD],
                            xt[:, ci, 128 * tt : 128 * tt + 128],
                            wv_sb[:, ci, :],
                            start=(ci == 0),
                            stop=(ci == NC128 - 1),
                        )
                    if cc == 3:
                        kt_idx = 4 * ts + tt
                        va_t = va_sb[:, kt_idx].rearrange("p (h j) -> p h j", j=65)
                        nc.vector.tensor_tensor(
                            va_t[:, :, 0:64],
                            box["ps"][:, 0:GD].rearrange("p (h j) -> p h j", j=64),
                            bv_bc[:].rearrange("p (h j) -> p h j", j=64),
                            mybir.AluOpType.add,
                        )
                        nc.vector.tensor_copy(va_t[:, :, 64], ones_sb[:])

                for tt in range(4):
                    box = {}
                    for cc in range(4):
                        units.append(lambda tt=tt, cc=cc, box=box: v_sub(tt, cc, box))

                return units

            def make_ph3_units(ts):
                units = []

                def po_unit(tt, nn):
                    po = psum.tile([128, 512], F32, tag="work")
                    for mm in range(2):
                        nc.tensor.matmul(
                            po[:],
                            yt_sb[:, mm, 128 * tt : 128 * tt + 128],
                            wp_sb[:, mm, 512 * nn : 512 * nn + 512],
                            start=(mm == 0),
                            stop=(mm == 1),
                        )
                    ot = outp.tile([128, 512], F32, tag="ot")
                    if nn == 0:
                        nc.vector.tensor_copy(ot[:], po[:])
                    else:
                        nc.scalar.copy(ot[:], po[:])
                    nc.sync.dma_start(
                        out[128 * tt : 128 * tt + 128, 512 * nn : 512 * nn + 512],
                        ot[:],
                    )

                for tt in range(4 * ts, 4 * ts + 4):
                    for nn in range(2):
                        units.append(lambda tt=tt, nn=nn: po_unit(tt, nn))
                return units

            from collections import deque

            pending = deque()

            def pump(n):
                for _ in range(n):
                    if pending:
                        pending.popleft()()

            def phase2(m, qi):
                """Attention for head pair (2m, 2m+1) on q-tile qi.

                Scores for both heads via two concurrent row-tiled K=64
                matmuls (array rows 0-63 / 64-127) into the two banks of one
                [128, 1024] PSUM tile. For diagonal k-tiles (i = kt-4qi >= 1)
                the first 128*i q-columns are fully masked and skipped
                everywhere. One background work unit is pumped per k-tile to
                fill the PE gap left by the ACT-bound exp chain.
                """
                q0 = 512 * qi
                nk = 4 * qi + 4
                pyA = psacc.tile([65, 512], F32, tag="pyA")
                pyB = psacc.tile([65, 512], F32, tag="pyB")

                def qlo(kt):
                    i = kt - 4 * qi
                    return 128 * i if i > 0 else 0

                pend = {}

                def emit_s(kt):
                    lo = qlo(kt)
                    ps = psum.tile([128, 1024], F32, tag="big")
                    for half in range(2):
                        po = 64 * half
                        nc.tensor.matmul(
                            ps[:, 512 * half + lo : 512 * half + 512],
                            kt_sb[po : po + 64, m, 128 * kt : 128 * kt + 128],
                            qt_sb[po : po + 64, m, q0 + lo : q0 + 512],
                            start=True,
                            stop=True,
                        )
                    pend[kt] = ps

                emit_s(0)
                if nk > 1:
                    emit_s(1)
                for kt in range(nk):
                    ps = pend.pop(kt)
                    lo = qlo(kt)
                    et = expp.tile([128, 1024], F32R, tag="exp")
                    if lo == 0:
                        nc.scalar.activation(et[:], ps[:], AF.Exp)
                    else:
                        for half in range(2):
                            sl = slice(512 * half + lo, 512 * half + 512)
                            nc.scalar.activation(et[:, sl], ps[:, sl], AF.Exp)
                    i = kt - 4 * qi
                    if i >= 0:  # diagonal tile: apply causal mask to both heads
                        o = 384 - 128 * i
                        nc.vector.tensor_tensor(
                            et[:].rearrange("p (h q) -> p h q", h=2)[:, :, lo:512],
                            et[:].rearrange("p (h q) -> p h q", h=2)[:, :, lo:512],
                            maskw_sb[:, None, o + lo : o + 512].to_broadcast(
                                [128, 2, 512 - lo]
                            ),
                            mybir.AluOpType.mult,
                        )
                    for half, py in ((0, pyA), (1, pyB)):
                        h = 2 * m + half
                        nc.tensor.matmul(
                            py[:, lo:512],
                            va_sb[:, kt, 65 * h : 65 * h + 65],
                            et[:, 512 * half + lo : 512 * half + 512],
                            start=(kt == 0),
                            stop=(kt == nk - 1),
                        )
                    if kt + 2 < nk:
                        emit_s(kt + 2)
                    pump(1)

                for half, py in ((0, pyA), (1, pyB)):
                    # 1/den = exp(-ln(den)) on ACT (DVE reciprocal is 8 cyc/elem)
                    rec = miscp.tile([1, 512], F32, tag="rec")
                    nc.scalar.activation(rec[:], py[64:65, :], AF.Ln)
                    nc.scalar.activation(rec[:], rec[:], AF.Exp, scale=-1.0)
                    bc = miscp.tile([64, 512], F32, tag="bc")
                    nc.gpsimd.partition_broadcast(bc[:], rec[:])
                    nc.vector.tensor_mul(
                        yt_sb[64 * half : 64 * half + 64, m, q0 : q0 + 512],
                        py[0:64, :],
                        bc[:],
                    )
                pump(2)

            # Interleave: q-tile qi's attention needs only t-slices <= qi, so
            # t-slice ts+1's phase-1 work and t-slice ts's projection are
            # pumped into phase-2(ts)'s PE gaps; leftovers drain between.
            xh = issue_xn(0)
            for u in make_ph1_units(0, xh):
                u()
            for ts in range(NT512):
                if ts + 1 < NT512:
                    xh = issue_xn(ts + 1)
                    pending.extend(make_ph1_units(ts + 1, xh))
                for m in range(2):
                    phase2(m, ts)
                while pending:
                    pump(1)
                pending.extend(make_ph3_units(ts))
            while pending:
                pump(1)

    nc.compile()
    return nc


def _make_maskw():
    p = np.arange(128)[:, None]
    jj = np.arange(896)[None, :]
    return (jj >= p + 384).astype(np.float32)


def kernel(x, w_qkv, b_qkv, w_proj, b_proj, _trace=False):
    x = np.asarray(x, dtype=np.float32)
    w_qkv = np.asarray(w_qkv, dtype=np.float32)
    b_qkv = np.asarray(b_qkv, dtype=np.float32)
    w_proj = np.asarray(w_proj, dtype=np.float32)
    b_proj = np.asarray(b_proj, dtype=np.float32)

    if "nc" not in _CACHE:
        _CACHE["nc"] = _build()
    nc = _CACHE["nc"]

    maskw = _make_maskw()
    ones = np.ones((128, HPG), np.float32)
    ident = np.eye(128, dtype=np.float32)
    scale = 1.0 / np.sqrt(D)

    in_maps = []
    for core in range(NCORES):
        b, hg = core // (NCORES // B), core % (NCORES // B)
        cs = slice(GD * hg, GD * hg + GD)  # this core's head columns / dims
        in_maps.append(
            {
                "x": np.ascontiguousarray(x[b]),
                "wq": np.ascontiguousarray(w_qkv[:, 0:C][:, cs]) * scale,
                "wk": np.ascontiguousarray(w_qkv[:, C : 2 * C][:, cs]),
                "wv": np.ascontiguousarray(w_qkv[:, 2 * C : 3 * C][:, cs]),
                "wp": np.ascontiguousarray(w_proj[cs, :]),
                "bq": np.ascontiguousarray(b_qkv[0:C][cs]) * scale,
                "bk": np.ascontiguousarray(b_qkv[C : 2 * C][cs]),
                "bv": np.ascontiguousarray(b_qkv[2 * C : 3 * C][cs])[None, :],
                "ident": ident,
                "maskw": maskw,
                "ones4": ones,
            }
        )

    if _trace:
        _install_ntff_hook()
    res = run_bass_kernel_spmd(
        nc, in_maps, core_ids=list(range(NCORES)), trace=bool(_trace)
    )
    _CACHE["last_result"] = res

    out = np.zeros((B, T, C), np.float32)
    for b in range(B):
        acc = res.results[4 * b + 0]["out"].astype(np.float64)
        for i in range(1, NCORES // B):
            acc += res.results[4 * b + i]["out"]
        out[b] = (acc + b_proj).astype(np.float32)
    return out
